# revision 13
# baseline (speedup 1.0000x reference)
"""Causal attention with ALiBi for nn_CausalAttention (B=4, T=2048, C=1024,
16 heads) on 8 TRN2 NeuronCores.

Sharding: batch (4) x head-group (2 groups of 8 heads) -> 8 cores.
Each core computes, for its batch b and head group g:
  qT/kT = (Wg.T @ x.T) projections in [d, t] layout, v in [t, d] layout,
  per head: sT[j, i] = qk/8 + slope*(j - i) via an augmented one-hot bias
  matmul (per-column -slope*i, numerically cancels in softmax) plus an ACT
  exp bias of +slope*j (exact fp32); causal masking by gpsimd affine_select
  (also kills Inf from masked overflow); PV with an appended ones column
  gives the softmax denominator; normalization via vector.reciprocal +
  gpsimd partition_broadcast; final y_partial = oT.T @ Wo_rows.
Host sums the two head-group partials per batch.

Matmuls run in float32r (TF32-like, ~1e-3 rel); probs/V in bf16.
"""

import math

import numpy as np

import concourse.bass as bass
import concourse.mybir as mybir
import concourse.tile as tile
from concourse import bacc
from concourse.bass_utils import run_bass_kernel_spmd

B, T, C = 4, 2048, 1024
NH, HD = 16, 64
NHC = 8  # heads per core
BLOCK_SIZE = 2048
NJB = T // 128  # 16 j-blocks
NCH = T // 512  # 4 i-chunks
P = 128

f32 = mybir.dt.float32
f32r = mybir.dt.float32r
bf16 = mybir.dt.bfloat16

LAST_RESULTS = None
_NC_CACHE = None


def get_slopes(n):
    def pow2(n):
        start = 2 ** (-(2 ** (-(math.log2(n) - 3))))
        return [start * start**i for i in range(n)]

    if math.log2(n).is_integer():
        return pow2(n)
    c = 2 ** math.floor(math.log2(n))
    return pow2(c) + get_slopes(2 * c)[0::2][: n - c]


# compact pT tile index: tiles (jb, c) with c >= jb//4
_PT_OFFS = []
_o = 0
for _jb in range(NJB):
    _PT_OFFS.append(_o)
    _o += NCH - _jb // 4
NPT = _o  # 40


def build_kernel():
    nc = bacc.Bacc("TRN2", target_bir_lowering=False, debug=False, num_devices=8)

    xT_d = nc.dram_tensor("xT", [C, T], f32, kind="ExternalInput").ap()
    wq_d = nc.dram_tensor("wq", [C, 512], f32, kind="ExternalInput").ap()
    wk_d = nc.dram_tensor("wk", [C, 512], f32, kind="ExternalInput").ap()
    wv_d = nc.dram_tensor("wv", [C, 512], f32, kind="ExternalInput").ap()
    wo_d = nc.dram_tensor("wo", [512, C], f32, kind="ExternalInput").ap()
    qaug_d = nc.dram_tensor("qaug", [NHC, T], f32, kind="ExternalInput").ap()
    onehot_d = nc.dram_tensor("onehot", [NHC, NHC, P], f32, kind="ExternalInput").ap()
    biasj_d = nc.dram_tensor("biasj", [P, NHC, NJB], f32, kind="ExternalInput").ap()
    y_d = nc.dram_tensor("y", [T, C], f32, kind="ExternalOutput").ap()

    xT_r = xT_d.rearrange("(cb p) t -> p cb t", p=P)  # [128, 8, 2048]
    wq_r = wq_d.rearrange("(cb p) m -> p cb m", p=P)  # [128, 8, 512]
    wk_r = wk_d.rearrange("(cb p) m -> p cb m", p=P)
    wv_r = wv_d.rearrange("(cb p) m -> p cb m", p=P)
    wo_r = wo_d.rearrange("(mb p) n -> p mb n", p=P)  # [128, 4, 1024]
    y_r = y_d.rearrange("(tb p) c -> p tb c", p=P)  # [128, 16, 1024]

    with tile.TileContext(nc) as tc:
        with (
            tc.tile_pool(name="persist", bufs=1) as persist,
            tc.tile_pool(name="work", bufs=2) as work,
            tc.tile_pool(name="psA", bufs=1, space="PSUM") as psA,
            tc.tile_pool(name="psB", bufs=2, space="PSUM") as psB,
            tc.tile_pool(name="psC", bufs=2, space="PSUM") as psC,
        ):
            # ---- persistent tiles ----
            qTr = persist.tile([P, 4, T], f32r)  # c' = 128*m + p ; head = c'//64
            kTr = persist.tile([P, 4, T], f32r)
            vaug = persist.tile([P, NJB, NHC, 66], bf16)
            oT = persist.tile([P, 4, T], f32r)
            qaugr = persist.tile([NHC, T], f32r)
            onehotr = persist.tile([NHC, NHC, P], f32r)
            biasj = persist.tile([P, NHC, NJB], f32)

            nc.gpsimd.memset(vaug[:, :, :, 64:66], 1.0)
            nc.sync.dma_start(biasj[:], biasj_d[:])

            # ---- projection pass 1: qT, kT ----
            with (
                tc.tile_pool(name="aux", bufs=1) as aux,
                tc.tile_pool(name="wqk", bufs=1) as wqk,
                tc.tile_pool(name="wst", bufs=2) as wst,
                tc.tile_pool(name="xp1", bufs=2) as xp1,
                tc.tile_pool(name="xr1", bufs=8) as xr1,
            ):
                aug32 = aux.tile([NHC, T], f32, tag="aug32")
                nc.sync.dma_start(aug32[:], qaug_d[:])
                nc.vector.tensor_copy(qaugr[:], aug32[:])
                oh32 = aux.tile([NHC, NHC, P], f32, tag="oh32")
                nc.sync.dma_start(oh32[:], onehot_d[:])
                nc.vector.tensor_copy(onehotr[:], oh32[:])

                wqr = wqk.tile([P, 8, 512], f32r)
                wkr = wqk.tile([P, 8, 512], f32r)
                for w_dram, w_rnd in ((wq_r, wqr), (wk_r, wkr)):
                    for c in range(8):
                        w32 = wst.tile([P, 512], f32, tag="w32")
                        nc.sync.dma_start(w32[:], w_dram[:, c, :])
                        nc.vector.tensor_copy(w_rnd[:, c, :], w32[:])

                for tck in range(NCH):  # t-chunks of 512
                    xts = []
                    for c in range(8):
                        x32 = xp1.tile([P, 512], f32, tag="x32")
                        nc.sync.dma_start(x32[:], xT_r[:, c, bass.ts(tck, 512)])
                        xtr = xr1.tile([P, 512], f32r, tag="xtr")
                        nc.vector.tensor_copy(xtr[:], x32[:])
                        xts.append(xtr)
                    for m in range(4):
                        psq = psB.tile([P, 512], f32, tag="pb")
                        psk = psB.tile([P, 512], f32, tag="pb")
                        for c in range(8):
                            nc.tensor.matmul(
                                psq[:],
                                wqr[:, c, bass.ts(m, P)],
                                xts[c][:],
                                start=(c == 0),
                                stop=(c == 7),
                            )
                            nc.tensor.matmul(
                                psk[:],
                                wkr[:, c, bass.ts(m, P)],
                                xts[c][:],
                                start=(c == 0),
                                stop=(c == 7),
                            )
                        nc.vector.tensor_copy(
                            qTr[:, m, bass.ts(tck, 512)], psq[:]
                        )
                        nc.vector.tensor_copy(
                            kTr[:, m, bass.ts(tck, 512)], psk[:]
                        )

            # ---- projection pass 2: v ----
            with (
                tc.tile_pool(name="wvp", bufs=1) as wvp,
                tc.tile_pool(name="wst2", bufs=2) as wst2,
                tc.tile_pool(name="xp2", bufs=2) as xp2,
                tc.tile_pool(name="xr2", bufs=8) as xr2,
            ):
                wvr = wvp.tile([P, 8, 512], f32r)
                for c in range(8):
                    w32 = wst2.tile([P, 512], f32, tag="w32")
                    nc.sync.dma_start(w32[:], wv_r[:, c, :])
                    nc.vector.tensor_copy(wvr[:, c, :], w32[:])

                for tck in range(NCH):
                    xts = []
                    for c in range(8):
                        x32 = xp2.tile([P, 512], f32, tag="x32")
                        nc.sync.dma_start(x32[:], xT_r[:, c, bass.ts(tck, 512)])
                        xtr = xr2.tile([P, 512], f32r, tag="xtr")
                        nc.vector.tensor_copy(xtr[:], x32[:])
                        xts.append(xtr)
                    for tb in range(4):
                        psv = psB.tile([P, 512], f32, tag="pb")
                        for c in range(8):
                            nc.tensor.matmul(
                                psv[:],
                                xts[c][:, bass.ts(tb, P)],
                                wvr[:, c, :],
                                start=(c == 0),
                                stop=(c == 7),
                            )
                        nc.vector.tensor_copy(
                            vaug[:, 4 * tck + tb, :, 0:64],
                            psv[:].rearrange("p (h d) -> p h d", h=NHC),
                        )

            # ---- attention ----
            ptp_cm = tc.tile_pool(name="ptp", bufs=1)
            ptp = ptp_cm.__enter__()
            for h in range(NHC):
                hp = (h % 2) * 64
                hm = h // 2
                pT = ptp.tile([P, NPT, 512], bf16, tag="pT")
                for jb in range(NJB):
                    c0 = jb // 4
                    nact = NCH - c0
                    idx0 = _PT_OFFS[jb]
                    ssum = psA.tile([P, 4, 512], f32, tag="ssum")
                    for ci in range(nact):
                        c = c0 + ci
                        nc.tensor.matmul(
                            ssum[:, c, :],
                            kTr[hp : hp + 64, hm, bass.ts(jb, P)],
                            qTr[hp : hp + 64, hm, bass.ts(c, 512)],
                            start=True,
                            stop=False,
                        )
                        nc.tensor.matmul(
                            ssum[:, c, :],
                            onehotr[:, h, :],
                            qaugr[:, bass.ts(c, 512)],
                            start=False,
                            stop=True,
                        )
                    nc.scalar.activation(
                        pT[:, idx0 : idx0 + nact, :],
                        ssum[:, c0:NCH, :],
                        mybir.ActivationFunctionType.Exp,
                        bias=biasj[:, h, jb : jb + 1],
                        scale=1.0,
                    )
                    # causal mask on the diagonal tile (c == c0):
                    # keep where i - j >= 0 ; i = 512*c0 + f, j = 128*jb + p
                    nc.gpsimd.affine_select(
                        pT[:, idx0, :],
                        pT[:, idx0, :],
                        pattern=[[1, 512]],
                        compare_op=mybir.AluOpType.is_ge,
                        fill=0.0,
                        base=512 * c0 - 128 * jb,
                        channel_multiplier=-1,
                    )

                for c in range(NCH):
                    pot = psC.tile([65, 512], f32, tag="pot")
                    njb = 4 * c + 4
                    for jb in range(njb):
                        nc.tensor.matmul(
                            pot[:],
                            vaug[:, jb, h, 0:65],
                            pT[:, _PT_OFFS[jb] + (c - jb // 4), :],
                            start=(jb == 0),
                            stop=(jb == njb - 1),
                        )
                    srecip = work.tile([1, 512], f32, tag="srecip")
                    nc.vector.reciprocal(srecip[:], pot[64:65, :])
                    bcast = work.tile([64, 512], f32, tag="bcast")
                    nc.gpsimd.partition_broadcast(bcast[:], srecip[:])
                    nc.vector.tensor_tensor(
                        oT[hp : hp + 64, hm, bass.ts(c, 512)],
                        pot[0:64, :],
                        bcast[:],
                        mybir.AluOpType.mult,
                    )

            ptp_cm.__exit__(None, None, None)

            # ---- output projection ----
            with (
                tc.tile_pool(name="wop", bufs=1) as wop,
                tc.tile_pool(name="wst3", bufs=2) as wst3,
            ):
                wor = wop.tile([P, 4, C], f32r)
                for m in range(4):
                    wo32 = wst3.tile([P, C], f32, tag="wo32")
                    nc.sync.dma_start(wo32[:], wo_r[:, m, :])
                    nc.vector.tensor_copy(wor[:, m, :], wo32[:])

                for tb in range(NJB):
                    for cc in range(2):
                        psy = psB.tile([P, 512], f32, tag="pb")
                        for m in range(4):
                            nc.tensor.matmul(
                                psy[:],
                                oT[:, m, bass.ts(tb, P)],
                                wor[:, m, bass.ts(cc, 512)],
                                start=(m == 0),
                                stop=(m == 3),
                            )
                        ysb = work.tile([P, 512], f32, tag="ysb")
                        nc.vector.tensor_copy(ysb[:], psy[:])
                        nc.sync.dma_start(y_r[:, tb, bass.ts(cc, 512)], ysb[:])

    nc.compile()
    return nc


def kernel(x, Wq, Wk, Wv, Wo):
    global LAST_RESULTS, _NC_CACHE
    x = np.asarray(x, dtype=np.float32)
    Wq = np.asarray(Wq, dtype=np.float32)
    Wk = np.asarray(Wk, dtype=np.float32)
    Wv = np.asarray(Wv, dtype=np.float32)
    Wo = np.asarray(Wo, dtype=np.float32)

    slopes = np.asarray(get_slopes(NH), dtype=np.float32)
    ii = np.arange(T, dtype=np.float64)
    pp = np.arange(P, dtype=np.float64)

    if _NC_CACHE is None:
        _NC_CACHE = build_kernel()
    nc = _NC_CACHE

    in_maps = []
    for core in range(8):
        b, g = core // 2, core % 2
        hsl = slice(g * 512, (g + 1) * 512)
        core_slopes = slopes[g * NHC : (g + 1) * NHC].astype(np.float64)
        qaug = (-core_slopes[:, None] * ii[None, :]).astype(np.float32)
        onehot = np.zeros((NHC, NHC, P), np.float32)
        for h in range(NHC):
            onehot[h, h, :] = 1.0
        biasj = np.zeros((P, NHC, NJB), np.float32)
        for h in range(NHC):
            for jb in range(NJB):
                biasj[:, h, jb] = (core_slopes[h] * (128 * jb + pp)).astype(np.float32)
        in_maps.append(
            {
                "xT": np.ascontiguousarray(x[b].T),
                "wq": np.ascontiguousarray(Wq[:, hsl]) * np.float32(0.125),
                "wk": np.ascontiguousarray(Wk[:, hsl]),
                "wv": np.ascontiguousarray(Wv[:, hsl]),
                "wo": np.ascontiguousarray(Wo[hsl, :]),
                "qaug": qaug,
                "onehot": onehot,
                "biasj": biasj,
            }
        )

    res = run_bass_kernel_spmd(nc, in_maps, list(range(8)))
    LAST_RESULTS = res
    out = np.empty((B, T, C), dtype=np.float32)
    for b in range(B):
        out[b] = res.results[2 * b]["y"] + res.results[2 * b + 1]["y"]
    return out


# revision 25
# speedup vs baseline: 1.3213x; 1.3213x over previous
"""Causal attention with ALiBi for nn_CausalAttention (B=4, T=2048, C=1024,
16 heads) on 8 TRN2 NeuronCores.

Sharding: batch (4) x head-group (2 groups of 8 heads) -> 8 cores.
Each core computes, for its batch b and head group g:
  qT/kT = (Wg.T @ x.T) projections in [d, t] layout, v in [t, d] layout,
  per head: sT[j, i] = qk/8 + slope*(j - i) via an augmented one-hot bias
  matmul (per-column -slope*i, numerically cancels in softmax) plus an ACT
  exp bias of +slope*j (exact fp32); causal masking by gpsimd affine_select
  (also kills Inf from masked overflow); PV with an appended ones column
  gives the softmax denominator; normalization via vector.reciprocal +
  gpsimd partition_broadcast; final y_partial = oT.T @ Wo_rows.
Host sums the two head-group partials per batch.

Matmuls run in float32r (TF32-like, ~1e-3 rel); probs/V in bf16.
"""

import math

import numpy as np

import concourse.bass as bass
import concourse.mybir as mybir
import concourse.tile as tile
from concourse import bacc
from concourse.bass_utils import run_bass_kernel_spmd

B, T, C = 4, 2048, 1024
NH, HD = 16, 64
NHC = 8  # heads per core
BLOCK_SIZE = 2048
NJB = T // 128  # 16 j-blocks
NCH = T // 512  # 4 i-chunks
P = 128

f32 = mybir.dt.float32
f32r = mybir.dt.float32r
bf16 = mybir.dt.bfloat16

LAST_RESULTS = None
_NC_CACHE = None


def get_slopes(n):
    def pow2(n):
        start = 2 ** (-(2 ** (-(math.log2(n) - 3))))
        return [start * start**i for i in range(n)]

    if math.log2(n).is_integer():
        return pow2(n)
    c = 2 ** math.floor(math.log2(n))
    return pow2(c) + get_slopes(2 * c)[0::2][: n - c]


# compact pT tile index: tiles (jb, c) with c >= jb//4
_PT_OFFS = []
_o = 0
for _jb in range(NJB):
    _PT_OFFS.append(_o)
    _o += NCH - _jb // 4
NPT = _o  # 40


def build_kernel():
    nc = bacc.Bacc("TRN2", target_bir_lowering=False, debug=False, num_devices=8)

    xT_d = nc.dram_tensor("xT", [C, T], f32, kind="ExternalInput").ap()
    wq_d = nc.dram_tensor("wq", [C, 512], f32, kind="ExternalInput").ap()
    wk_d = nc.dram_tensor("wk", [C, 512], f32, kind="ExternalInput").ap()
    wv_d = nc.dram_tensor("wv", [C, 512], f32, kind="ExternalInput").ap()
    wo_d = nc.dram_tensor("wo", [512, C], f32, kind="ExternalInput").ap()
    qaug_d = nc.dram_tensor("qaugb", [8, NHC, T], bf16, kind="ExternalInput").ap()
    kaug_d = nc.dram_tensor("kaugb", [8, NHC, T], bf16, kind="ExternalInput").ap()
    biasj_d = nc.dram_tensor("biasj", [P, NHC, NJB], f32, kind="ExternalInput").ap()
    y_d = nc.dram_tensor("y", [T, C], f32, kind="ExternalOutput").ap()

    xT_r = xT_d.rearrange("(cb p) t -> p cb t", p=P)  # [128, 8, 2048]
    wq_r = wq_d.rearrange("(cb p) m -> p cb m", p=P)  # [128, 8, 512]
    wk_r = wk_d.rearrange("(cb p) m -> p cb m", p=P)
    wv_r = wv_d.rearrange("(cb p) m -> p cb m", p=P)
    wo_r = wo_d.rearrange("(mb p) n -> p mb n", p=P)  # [128, 4, 1024]
    y_r = y_d.rearrange("(tb p) c -> p tb c", p=P)  # [128, 16, 1024]

    with tile.TileContext(nc) as tc:
        with (
            tc.tile_pool(name="persist", bufs=1) as persist,
            tc.tile_pool(name="work", bufs=2) as work,
            tc.tile_pool(name="psA", bufs=1, space="PSUM") as psA,
            tc.tile_pool(name="psB", bufs=2, space="PSUM") as psB,
            tc.tile_pool(name="psC", bufs=2, space="PSUM") as psC,
        ):
            # ---- persistent tiles ----
            # qT2/kT2: per head h, rows 0-63 = head data (d), rows 64-71 =
            # augmented bias rows; K=72 matmul contracts both at once.
            qT2 = persist.tile([72, NHC, T], bf16)
            kT2 = persist.tile([72, NHC, T], bf16)
            vaug = persist.tile([P, NJB, NHC, 66], bf16)
            oT = persist.tile([P, 4, T], f32r)
            biasj = persist.tile([P, NHC, NJB], f32)

            nc.gpsimd.memset(vaug[:, :, :, 64:66], 1.0)
            nc.sync.dma_start(biasj[:], biasj_d[:])
            # aug rows: kT2 row 64+r of head h is 1.0 iff r == h;
            # qT2 row 64+r of every head = -slope_r * i
            nc.sync.dma_start(kT2[64:72, :, :], kaug_d[:])
            nc.sync.dma_start(qT2[64:72, :, :], qaug_d[:])

            # ---- projection pass 1: qT, kT ----
            with (
                tc.tile_pool(name="aux", bufs=1) as aux,
                tc.tile_pool(name="wqk", bufs=1) as wqk,
                tc.tile_pool(name="wst", bufs=2) as wst,
                tc.tile_pool(name="xp1", bufs=2) as xp1,
                tc.tile_pool(name="xr1", bufs=8) as xr1,
            ):

                wqr = wqk.tile([P, 8, 512], f32r)
                wkr = wqk.tile([P, 8, 512], f32r)
                for w_dram, w_rnd in ((wq_r, wqr), (wk_r, wkr)):
                    for c in range(8):
                        w32 = wst.tile([P, 512], f32, tag="w32")
                        nc.sync.dma_start(w32[:], w_dram[:, c, :])
                        nc.vector.tensor_copy(w_rnd[:, c, :], w32[:])

                for tck in range(NCH):  # t-chunks of 512
                    xts = []
                    for c in range(8):
                        x32 = xp1.tile([P, 512], f32, tag="x32")
                        nc.sync.dma_start(x32[:], xT_r[:, c, bass.ts(tck, 512)])
                        xtr = xr1.tile([P, 512], f32r, tag="xtr")
                        nc.vector.tensor_copy(xtr[:], x32[:])
                        xts.append(xtr)
                    for m in range(4):
                        psq = psB.tile([P, 512], f32, tag="pb")
                        psk = psB.tile([P, 512], f32, tag="pb")
                        for c in range(8):
                            nc.tensor.matmul(
                                psq[:],
                                wqr[:, c, bass.ts(m, P)],
                                xts[c][:],
                                start=(c == 0),
                                stop=(c == 7),
                            )
                            nc.tensor.matmul(
                                psk[:],
                                wkr[:, c, bass.ts(m, P)],
                                xts[c][:],
                                start=(c == 0),
                                stop=(c == 7),
                            )
                        qstag = work.tile([P, 512], bf16, tag="qkstag")
                        kstag = work.tile([P, 512], bf16, tag="qkstag")
                        nc.vector.tensor_copy(qstag[:], psq[:])
                        nc.vector.tensor_copy(kstag[:], psk[:])
                        tsl = bass.ts(tck, 512)
                        nc.sync.dma_start(qT2[0:64, 2 * m, tsl], qstag[0:64, :])
                        nc.sync.dma_start(qT2[0:64, 2 * m + 1, tsl], qstag[64:128, :])
                        nc.sync.dma_start(kT2[0:64, 2 * m, tsl], kstag[0:64, :])
                        nc.sync.dma_start(kT2[0:64, 2 * m + 1, tsl], kstag[64:128, :])

            # ---- projection pass 2: v ----
            with (
                tc.tile_pool(name="wvp", bufs=1) as wvp,
                tc.tile_pool(name="wst2", bufs=2) as wst2,
                tc.tile_pool(name="xp2", bufs=2) as xp2,
                tc.tile_pool(name="xr2", bufs=8) as xr2,
            ):
                wvr = wvp.tile([P, 8, 512], f32r)
                for c in range(8):
                    w32 = wst2.tile([P, 512], f32, tag="w32")
                    nc.sync.dma_start(w32[:], wv_r[:, c, :])
                    nc.vector.tensor_copy(wvr[:, c, :], w32[:])

                for tck in range(NCH):
                    xts = []
                    for c in range(8):
                        x32 = xp2.tile([P, 512], f32, tag="x32")
                        nc.sync.dma_start(x32[:], xT_r[:, c, bass.ts(tck, 512)])
                        xtr = xr2.tile([P, 512], f32r, tag="xtr")
                        nc.vector.tensor_copy(xtr[:], x32[:])
                        xts.append(xtr)
                    for tb in range(4):
                        psv = psB.tile([P, 512], f32, tag="pb")
                        for c in range(8):
                            nc.tensor.matmul(
                                psv[:],
                                xts[c][:, bass.ts(tb, P)],
                                wvr[:, c, :],
                                start=(c == 0),
                                stop=(c == 7),
                            )
                        nc.vector.tensor_copy(
                            vaug[:, 4 * tck + tb, :, 0:64],
                            psv[:].rearrange("p (h d) -> p h d", h=NHC),
                        )

            # ---- attention ----
            ptp_cm = tc.tile_pool(name="ptp", bufs=1)
            ptp = ptp_cm.__enter__()
            for h in range(NHC):
                hp = (h % 2) * 64
                hm = h // 2
                pT = ptp.tile([P, NPT, 512], bf16, tag="pT")
                for jb in range(NJB):
                    c0 = jb // 4
                    nact = NCH - c0
                    idx0 = _PT_OFFS[jb]
                    ssum = psA.tile([P, 4, 512], f32, tag="ssum")
                    for ci in range(nact):
                        c = c0 + ci
                        nc.tensor.matmul(
                            ssum[:, c, :],
                            kT2[:, h, bass.ts(jb, P)],
                            qT2[:, h, bass.ts(c, 512)],
                            start=True,
                            stop=True,
                        )
                    nc.scalar.activation(
                        pT[:, idx0 : idx0 + nact, :],
                        ssum[:, c0:NCH, :],
                        mybir.ActivationFunctionType.Exp,
                        bias=biasj[:, h, jb : jb + 1],
                        scale=1.0,
                    )
                    # causal mask on the diagonal tile (c == c0):
                    # keep where i - j >= 0 ; i = 512*c0 + f, j = 128*jb + p
                    nc.gpsimd.affine_select(
                        pT[:, idx0, :],
                        pT[:, idx0, :],
                        pattern=[[1, 512]],
                        compare_op=mybir.AluOpType.is_ge,
                        fill=0.0,
                        base=512 * c0 - 128 * jb,
                        channel_multiplier=-1,
                    )

                for c in range(NCH):
                    pot = psC.tile([65, 512], f32, tag="pot")
                    njb = 4 * c + 4
                    for jb in range(njb):
                        nc.tensor.matmul(
                            pot[:],
                            vaug[:, jb, h, 0:65],
                            pT[:, _PT_OFFS[jb] + (c - jb // 4), :],
                            start=(jb == 0),
                            stop=(jb == njb - 1),
                        )
                    srecip = work.tile([1, 512], f32, tag="srecip")
                    nc.vector.reciprocal(srecip[:], pot[64:65, :])
                    bcast = work.tile([64, 512], f32, tag="bcast")
                    nc.gpsimd.partition_broadcast(bcast[:], srecip[:])
                    nc.vector.tensor_tensor(
                        oT[hp : hp + 64, hm, bass.ts(c, 512)],
                        pot[0:64, :],
                        bcast[:],
                        mybir.AluOpType.mult,
                    )

            ptp_cm.__exit__(None, None, None)

            # ---- output projection ----
            with (
                tc.tile_pool(name="wop", bufs=1) as wop,
                tc.tile_pool(name="wst3", bufs=2) as wst3,
            ):
                wor = wop.tile([P, 4, C], f32r)
                for m in range(4):
                    wo32 = wst3.tile([P, C], f32, tag="wo32")
                    nc.sync.dma_start(wo32[:], wo_r[:, m, :])
                    nc.vector.tensor_copy(wor[:, m, :], wo32[:])

                for tb in range(NJB):
                    for cc in range(2):
                        psy = psB.tile([P, 512], f32, tag="pb")
                        for m in range(4):
                            nc.tensor.matmul(
                                psy[:],
                                oT[:, m, bass.ts(tb, P)],
                                wor[:, m, bass.ts(cc, 512)],
                                start=(m == 0),
                                stop=(m == 3),
                            )
                        ysb = work.tile([P, 512], f32, tag="ysb")
                        nc.vector.tensor_copy(ysb[:], psy[:])
                        nc.sync.dma_start(y_r[:, tb, bass.ts(cc, 512)], ysb[:])

    nc.compile()
    return nc


def kernel(x, Wq, Wk, Wv, Wo):
    global LAST_RESULTS, _NC_CACHE
    x = np.asarray(x, dtype=np.float32)
    Wq = np.asarray(Wq, dtype=np.float32)
    Wk = np.asarray(Wk, dtype=np.float32)
    Wv = np.asarray(Wv, dtype=np.float32)
    Wo = np.asarray(Wo, dtype=np.float32)

    slopes = np.asarray(get_slopes(NH), dtype=np.float32)
    ii = np.arange(T, dtype=np.float64)
    pp = np.arange(P, dtype=np.float64)

    if _NC_CACHE is None:
        _NC_CACHE = build_kernel()
    nc = _NC_CACHE

    in_maps = []
    for core in range(8):
        b, g = core // 2, core % 2
        hsl = slice(g * 512, (g + 1) * 512)
        core_slopes = slopes[g * NHC : (g + 1) * NHC].astype(np.float64)
        import ml_dtypes

        qaug1 = (-core_slopes[:, None] * ii[None, :]).astype(ml_dtypes.bfloat16)
        qaugb = np.ascontiguousarray(
            np.broadcast_to(qaug1[:, None, :], (8, NHC, T))
        )
        kaugb = np.zeros((8, NHC, T), ml_dtypes.bfloat16)
        for h in range(NHC):
            kaugb[h, h, :] = ml_dtypes.bfloat16(1.0)
        biasj = np.zeros((P, NHC, NJB), np.float32)
        for h in range(NHC):
            for jb in range(NJB):
                biasj[:, h, jb] = (core_slopes[h] * (128 * jb + pp)).astype(np.float32)
        in_maps.append(
            {
                "xT": np.ascontiguousarray(x[b].T),
                "wq": np.ascontiguousarray(Wq[:, hsl]) * np.float32(0.125),
                "wk": np.ascontiguousarray(Wk[:, hsl]),
                "wv": np.ascontiguousarray(Wv[:, hsl]),
                "wo": np.ascontiguousarray(Wo[hsl, :]),
                "qaugb": qaugb,
                "kaugb": kaugb,
                "biasj": biasj,
            }
        )

    res = run_bass_kernel_spmd(nc, in_maps, list(range(8)))
    LAST_RESULTS = res
    out = np.empty((B, T, C), dtype=np.float32)
    for b in range(B):
        out[b] = res.results[2 * b]["y"] + res.results[2 * b + 1]["y"]
    return out


# revision 28
# speedup vs baseline: 1.4899x; 1.1277x over previous
"""Causal attention with ALiBi for nn_CausalAttention (B=4, T=2048, C=1024,
16 heads) on 8 TRN2 NeuronCores.

Sharding: batch (4) x head-group (2 groups of 8 heads) -> 8 cores.
Each core computes, for its batch b and head group g:
  qT/kT = (Wg.T @ x.T) projections in [d, t] layout, v in [t, d] layout,
  per head: sT[j, i] = qk/8 + slope*(j - i) via an augmented one-hot bias
  matmul (per-column -slope*i, numerically cancels in softmax) plus an ACT
  exp bias of +slope*j (exact fp32); causal masking by gpsimd affine_select
  (also kills Inf from masked overflow); PV with an appended ones column
  gives the softmax denominator; normalization via vector.reciprocal +
  gpsimd partition_broadcast; final y_partial = oT.T @ Wo_rows.
Host sums the two head-group partials per batch.

Matmuls run in float32r (TF32-like, ~1e-3 rel); probs/V in bf16.
"""

import math

import numpy as np

import concourse.bass as bass
import concourse.mybir as mybir
import concourse.tile as tile
from concourse import bacc
from concourse.bass_utils import run_bass_kernel_spmd

B, T, C = 4, 2048, 1024
NH, HD = 16, 64
NHC = 8  # heads per core
BLOCK_SIZE = 2048
NJB = T // 128  # 16 j-blocks
NCH = T // 512  # 4 i-chunks
P = 128

f32 = mybir.dt.float32
f32r = mybir.dt.float32r
bf16 = mybir.dt.bfloat16

LAST_RESULTS = None
_NC_CACHE = None


def get_slopes(n):
    def pow2(n):
        start = 2 ** (-(2 ** (-(math.log2(n) - 3))))
        return [start * start**i for i in range(n)]

    if math.log2(n).is_integer():
        return pow2(n)
    c = 2 ** math.floor(math.log2(n))
    return pow2(c) + get_slopes(2 * c)[0::2][: n - c]


# compact pT tile index: tiles (jb, c) with c >= jb//4
_PT_OFFS = []
_o = 0
for _jb in range(NJB):
    _PT_OFFS.append(_o)
    _o += NCH - _jb // 4
NPT = _o  # 40


def build_kernel():
    nc = bacc.Bacc("TRN2", target_bir_lowering=False, debug=False, num_devices=8)

    xT_d = nc.dram_tensor("xT", [C, T], f32, kind="ExternalInput").ap()
    wq_d = nc.dram_tensor("wq", [C, 512], f32, kind="ExternalInput").ap()
    wk_d = nc.dram_tensor("wk", [C, 512], f32, kind="ExternalInput").ap()
    wv_d = nc.dram_tensor("wv", [C, 512], f32, kind="ExternalInput").ap()
    wo_d = nc.dram_tensor("wo", [512, C], f32, kind="ExternalInput").ap()
    qaug_d = nc.dram_tensor("qaugb", [8, NHC, T], bf16, kind="ExternalInput").ap()
    kaug_d = nc.dram_tensor("kaugb", [8, NHC, T], bf16, kind="ExternalInput").ap()
    biasj_d = nc.dram_tensor("biasj", [P, NHC, NJB], f32, kind="ExternalInput").ap()
    y_d = nc.dram_tensor("y", [T, C], f32, kind="ExternalOutput").ap()

    xT_r = xT_d.rearrange("(cb p) t -> p cb t", p=P)  # [128, 8, 2048]
    wq_r = wq_d.rearrange("(cb p) m -> p cb m", p=P)  # [128, 8, 512]
    wk_r = wk_d.rearrange("(cb p) m -> p cb m", p=P)
    wv_r = wv_d.rearrange("(cb p) m -> p cb m", p=P)
    wo_r = wo_d.rearrange("(mb p) n -> p mb n", p=P)  # [128, 4, 1024]
    y_r = y_d.rearrange("(tb p) c -> p tb c", p=P)  # [128, 16, 1024]

    with tile.TileContext(nc) as tc:
        with (
            tc.tile_pool(name="persist", bufs=1) as persist,
            tc.tile_pool(name="work", bufs=2) as work,
            tc.tile_pool(name="psA", bufs=2, space="PSUM") as psA,
            tc.tile_pool(name="psB", bufs=2, space="PSUM") as psB,
            tc.tile_pool(name="psC", bufs=2, space="PSUM") as psC,
        ):
            # ---- persistent tiles ----
            # qT2/kT2: per head h, rows 0-63 = head data (d), rows 64-71 =
            # augmented bias rows; K=72 matmul contracts both at once.
            qT2 = persist.tile([72, NHC, T], bf16)
            kT2 = persist.tile([72, NHC, T], bf16)
            vaug = persist.tile([P, NJB, NHC, 66], bf16)
            oT = persist.tile([P, 4, T], f32r)
            biasj = persist.tile([P, NHC, NJB], f32)

            nc.gpsimd.memset(vaug[:, :, :, 64:66], 1.0)
            nc.sync.dma_start(biasj[:], biasj_d[:])
            # aug rows: kT2 row 64+r of head h is 1.0 iff r == h;
            # qT2 row 64+r of every head = -slope_r * i
            nc.sync.dma_start(kT2[64:72, :, :], kaug_d[:])
            nc.sync.dma_start(qT2[64:72, :, :], qaug_d[:])

            # ---- projection pass 1: qT, kT ----
            with (
                tc.tile_pool(name="aux", bufs=1) as aux,
                tc.tile_pool(name="wqk", bufs=1) as wqk,
                tc.tile_pool(name="wst", bufs=2) as wst,
                tc.tile_pool(name="xp1", bufs=2) as xp1,
                tc.tile_pool(name="xr1", bufs=8) as xr1,
            ):

                wqr = wqk.tile([P, 8, 512], bf16)
                wkr = wqk.tile([P, 8, 512], bf16)
                for w_dram, w_rnd in ((wq_r, wqr), (wk_r, wkr)):
                    for c in range(8):
                        w32 = wst.tile([P, 512], f32, tag="w32")
                        nc.sync.dma_start(w32[:], w_dram[:, c, :])
                        nc.vector.tensor_copy(w_rnd[:, c, :], w32[:])

                for tck in range(NCH):  # t-chunks of 512
                    xts = []
                    for c in range(8):
                        x32 = xp1.tile([P, 512], f32, tag="x32")
                        nc.sync.dma_start(x32[:], xT_r[:, c, bass.ts(tck, 512)])
                        xtr = xr1.tile([P, 512], bf16, tag="xtr")
                        nc.vector.tensor_copy(xtr[:], x32[:])
                        xts.append(xtr)
                    for m in range(4):
                        psq = psB.tile([P, 512], f32, tag="pb")
                        psk = psB.tile([P, 512], f32, tag="pb")
                        for c in range(8):
                            nc.tensor.matmul(
                                psq[:],
                                wqr[:, c, bass.ts(m, P)],
                                xts[c][:],
                                start=(c == 0),
                                stop=(c == 7),
                            )
                            nc.tensor.matmul(
                                psk[:],
                                wkr[:, c, bass.ts(m, P)],
                                xts[c][:],
                                start=(c == 0),
                                stop=(c == 7),
                            )
                        qstag = work.tile([P, 512], bf16, tag="qkstag")
                        kstag = work.tile([P, 512], bf16, tag="qkstag")
                        nc.vector.tensor_copy(qstag[:], psq[:])
                        nc.vector.tensor_copy(kstag[:], psk[:])
                        tsl = bass.ts(tck, 512)
                        nc.sync.dma_start(qT2[0:64, 2 * m, tsl], qstag[0:64, :])
                        nc.sync.dma_start(qT2[0:64, 2 * m + 1, tsl], qstag[64:128, :])
                        nc.sync.dma_start(kT2[0:64, 2 * m, tsl], kstag[0:64, :])
                        nc.sync.dma_start(kT2[0:64, 2 * m + 1, tsl], kstag[64:128, :])

            # ---- projection pass 2: v ----
            with (
                tc.tile_pool(name="wvp", bufs=1) as wvp,
                tc.tile_pool(name="wst2", bufs=2) as wst2,
                tc.tile_pool(name="xp2", bufs=2) as xp2,
                tc.tile_pool(name="xr2", bufs=8) as xr2,
            ):
                wvr = wvp.tile([P, 8, 512], bf16)
                for c in range(8):
                    w32 = wst2.tile([P, 512], f32, tag="w32")
                    nc.sync.dma_start(w32[:], wv_r[:, c, :])
                    nc.vector.tensor_copy(wvr[:, c, :], w32[:])

                for tck in range(NCH):
                    xts = []
                    for c in range(8):
                        x32 = xp2.tile([P, 512], f32, tag="x32")
                        nc.sync.dma_start(x32[:], xT_r[:, c, bass.ts(tck, 512)])
                        xtr = xr2.tile([P, 512], bf16, tag="xtr")
                        nc.vector.tensor_copy(xtr[:], x32[:])
                        xts.append(xtr)
                    for tb in range(4):
                        psv = psB.tile([P, 512], f32, tag="pb")
                        for c in range(8):
                            nc.tensor.matmul(
                                psv[:],
                                xts[c][:, bass.ts(tb, P)],
                                wvr[:, c, :],
                                start=(c == 0),
                                stop=(c == 7),
                            )
                        nc.vector.tensor_copy(
                            vaug[:, 4 * tck + tb, :, 0:64],
                            psv[:].rearrange("p (h d) -> p h d", h=NHC),
                        )

            # ---- attention ----
            ptp_cm = tc.tile_pool(name="ptp", bufs=2)
            ptp = ptp_cm.__enter__()
            for h in range(NHC):
                hp = (h % 2) * 64
                hm = h // 2
                pT = ptp.tile([P, NPT, 512], bf16, tag="pT")
                for jb in range(NJB):
                    c0 = jb // 4
                    nact = NCH - c0
                    idx0 = _PT_OFFS[jb]
                    # sub-groups of <=2 chunks so QK can run ahead of exp
                    for g0 in range(0, nact, 2):
                        ng = min(2, nact - g0)
                        ssum = psA.tile([P, 2, 512], f32, tag="ssum")
                        for ci in range(ng):
                            c = c0 + g0 + ci
                            nc.tensor.matmul(
                                ssum[:, ci, :],
                                kT2[:, h, bass.ts(jb, P)],
                                qT2[:, h, bass.ts(c, 512)],
                                start=True,
                                stop=True,
                            )
                        nc.scalar.activation(
                            pT[:, idx0 + g0 : idx0 + g0 + ng, :],
                            ssum[:, 0:ng, :],
                            mybir.ActivationFunctionType.Exp,
                            bias=biasj[:, h, jb : jb + 1],
                            scale=1.0,
                        )
                    # causal mask on the diagonal tile (c == c0):
                    # keep where i - j >= 0 ; i = 512*c0 + f, j = 128*jb + p
                    nc.gpsimd.affine_select(
                        pT[:, idx0, :],
                        pT[:, idx0, :],
                        pattern=[[1, 512]],
                        compare_op=mybir.AluOpType.is_ge,
                        fill=0.0,
                        base=512 * c0 - 128 * jb,
                        channel_multiplier=-1,
                    )

                for c in range(NCH):
                    pot = psC.tile([65, 512], f32, tag="pot")
                    njb = 4 * c + 4
                    for jb in range(njb):
                        nc.tensor.matmul(
                            pot[:],
                            vaug[:, jb, h, 0:65],
                            pT[:, _PT_OFFS[jb] + (c - jb // 4), :],
                            start=(jb == 0),
                            stop=(jb == njb - 1),
                        )
                    srecip = work.tile([1, 512], f32, tag="srecip")
                    nc.vector.reciprocal(srecip[:], pot[64:65, :])
                    bcast = work.tile([64, 512], f32, tag="bcast")
                    nc.gpsimd.partition_broadcast(bcast[:], srecip[:])
                    nc.vector.tensor_tensor(
                        oT[hp : hp + 64, hm, bass.ts(c, 512)],
                        pot[0:64, :],
                        bcast[:],
                        mybir.AluOpType.mult,
                    )

            ptp_cm.__exit__(None, None, None)

            # ---- output projection ----
            with (
                tc.tile_pool(name="wop", bufs=1) as wop,
                tc.tile_pool(name="wst3", bufs=2) as wst3,
            ):
                wor = wop.tile([P, 4, C], f32r)
                for m in range(4):
                    wo32 = wst3.tile([P, C], f32, tag="wo32")
                    nc.sync.dma_start(wo32[:], wo_r[:, m, :])
                    nc.vector.tensor_copy(wor[:, m, :], wo32[:])

                for tb in range(NJB):
                    for cc in range(2):
                        psy = psB.tile([P, 512], f32, tag="pb")
                        for m in range(4):
                            nc.tensor.matmul(
                                psy[:],
                                oT[:, m, bass.ts(tb, P)],
                                wor[:, m, bass.ts(cc, 512)],
                                start=(m == 0),
                                stop=(m == 3),
                            )
                        ysb = work.tile([P, 512], f32, tag="ysb")
                        nc.vector.tensor_copy(ysb[:], psy[:])
                        nc.sync.dma_start(y_r[:, tb, bass.ts(cc, 512)], ysb[:])

    nc.compile()
    return nc


def kernel(x, Wq, Wk, Wv, Wo):
    global LAST_RESULTS, _NC_CACHE
    x = np.asarray(x, dtype=np.float32)
    Wq = np.asarray(Wq, dtype=np.float32)
    Wk = np.asarray(Wk, dtype=np.float32)
    Wv = np.asarray(Wv, dtype=np.float32)
    Wo = np.asarray(Wo, dtype=np.float32)

    slopes = np.asarray(get_slopes(NH), dtype=np.float32)
    ii = np.arange(T, dtype=np.float64)
    pp = np.arange(P, dtype=np.float64)

    if _NC_CACHE is None:
        _NC_CACHE = build_kernel()
    nc = _NC_CACHE

    in_maps = []
    for core in range(8):
        b, g = core // 2, core % 2
        hsl = slice(g * 512, (g + 1) * 512)
        core_slopes = slopes[g * NHC : (g + 1) * NHC].astype(np.float64)
        import ml_dtypes

        qaug1 = (-core_slopes[:, None] * ii[None, :]).astype(ml_dtypes.bfloat16)
        qaugb = np.ascontiguousarray(
            np.broadcast_to(qaug1[:, None, :], (8, NHC, T))
        )
        kaugb = np.zeros((8, NHC, T), ml_dtypes.bfloat16)
        for h in range(NHC):
            kaugb[h, h, :] = ml_dtypes.bfloat16(1.0)
        biasj = np.zeros((P, NHC, NJB), np.float32)
        for h in range(NHC):
            for jb in range(NJB):
                biasj[:, h, jb] = (core_slopes[h] * (128 * jb + pp)).astype(np.float32)
        in_maps.append(
            {
                "xT": np.ascontiguousarray(x[b].T),
                "wq": np.ascontiguousarray(Wq[:, hsl]) * np.float32(0.125),
                "wk": np.ascontiguousarray(Wk[:, hsl]),
                "wv": np.ascontiguousarray(Wv[:, hsl]),
                "wo": np.ascontiguousarray(Wo[hsl, :]),
                "qaugb": qaugb,
                "kaugb": kaugb,
                "biasj": biasj,
            }
        )

    res = run_bass_kernel_spmd(nc, in_maps, list(range(8)))
    LAST_RESULTS = res
    out = np.empty((B, T, C), dtype=np.float32)
    for b in range(B):
        out[b] = res.results[2 * b]["y"] + res.results[2 * b + 1]["y"]
    return out


# revision 34
# speedup vs baseline: 1.8443x; 1.2378x over previous
"""Causal attention with ALiBi for nn_CausalAttention (B=4, T=2048, C=1024,
16 heads) on 8 TRN2 NeuronCores.

Sharding: batch (4) x head-group (2 groups of 8 heads) -> 8 cores.
Each core computes, for its batch b and head group g:
  qT/kT = (Wg.T @ x.T) projections in [d, t] layout, v in [t, d] layout,
  per head: sT[j, i] = qk/8 + slope*(j - i) via an augmented one-hot bias
  matmul (per-column -slope*i, numerically cancels in softmax) plus an ACT
  exp bias of +slope*j (exact fp32); causal masking by gpsimd affine_select
  (also kills Inf from masked overflow); PV with an appended ones column
  gives the softmax denominator; normalization via vector.reciprocal +
  gpsimd partition_broadcast; final y_partial = oT.T @ Wo_rows.
Host sums the two head-group partials per batch.

Matmuls run in float32r (TF32-like, ~1e-3 rel); probs/V in bf16.
"""

import math

import numpy as np

import concourse.bass as bass
import concourse.mybir as mybir
import concourse.tile as tile
from concourse import bacc
from concourse.bass_utils import run_bass_kernel_spmd

B, T, C = 4, 2048, 1024
NH, HD = 16, 64
NHC = 8  # heads per core
BLOCK_SIZE = 2048
NJB = T // 128  # 16 j-blocks
NCH = T // 512  # 4 i-chunks
P = 128

f32 = mybir.dt.float32
f32r = mybir.dt.float32r
bf16 = mybir.dt.bfloat16

LAST_RESULTS = None
_NC_CACHE = None


def get_slopes(n):
    def pow2(n):
        start = 2 ** (-(2 ** (-(math.log2(n) - 3))))
        return [start * start**i for i in range(n)]

    if math.log2(n).is_integer():
        return pow2(n)
    c = 2 ** math.floor(math.log2(n))
    return pow2(c) + get_slopes(2 * c)[0::2][: n - c]


# compact pT tile index: tiles (jb, c) with c >= jb//4
_PT_OFFS = []
_o = 0
for _jb in range(NJB):
    _PT_OFFS.append(_o)
    _o += NCH - _jb // 4
NPT = _o  # 40


def build_kernel():
    nc = bacc.Bacc("TRN2", target_bir_lowering=False, debug=False, num_devices=8)

    xT_d = nc.dram_tensor("xT", [C, T], f32, kind="ExternalInput").ap()
    wq_d = nc.dram_tensor("wq", [C, 512], f32, kind="ExternalInput").ap()
    wk_d = nc.dram_tensor("wk", [C, 512], f32, kind="ExternalInput").ap()
    wv_d = nc.dram_tensor("wv", [C, 512], f32, kind="ExternalInput").ap()
    wo_d = nc.dram_tensor("wo", [512, C], f32, kind="ExternalInput").ap()
    qaug_d = nc.dram_tensor("qaugb", [8, NHC, T], bf16, kind="ExternalInput").ap()
    kaug_d = nc.dram_tensor("kaugb", [8, NHC, T], bf16, kind="ExternalInput").ap()
    biasj_d = nc.dram_tensor("biasj", [P, NHC, NJB], f32, kind="ExternalInput").ap()
    y_d = nc.dram_tensor("y", [T, C], f32, kind="ExternalOutput").ap()

    xT_r = xT_d.rearrange("(cb p) t -> p cb t", p=P)  # [128, 8, 2048]
    wq_r = wq_d.rearrange("(cb p) m -> p cb m", p=P)  # [128, 8, 512]
    wk_r = wk_d.rearrange("(cb p) m -> p cb m", p=P)
    wv_r = wv_d.rearrange("(cb p) m -> p cb m", p=P)
    wo_r = wo_d.rearrange("(mb p) n -> p mb n", p=P)  # [128, 4, 1024]
    y_r = y_d.rearrange("(tb p) c -> p tb c", p=P)  # [128, 16, 1024]

    with tile.TileContext(nc) as tc:
        with (
            tc.tile_pool(name="persist", bufs=1) as persist,
            tc.tile_pool(name="work", bufs=2) as work,
            tc.tile_pool(name="psA", bufs=2, space="PSUM") as psA,
            tc.tile_pool(name="psB", bufs=2, space="PSUM") as psB,
            tc.tile_pool(name="psC", bufs=2, space="PSUM") as psC,
        ):
            # ---- persistent tiles ----
            # qT2/kT2: per head h, rows 0-63 = head data (d), rows 64-71 =
            # augmented bias rows; K=72 matmul contracts both at once.
            qT2 = persist.tile([72, NHC, T], bf16)
            kT2 = persist.tile([72, NHC, T], bf16)
            vaug = persist.tile([P, NJB, NHC, 66], bf16)
            oT = persist.tile([P, 4, T], bf16)
            biasj = persist.tile([P, NHC, NJB], f32)

            nc.gpsimd.memset(vaug[:, :, :, 64:66], 1.0)
            nc.sync.dma_start(biasj[:], biasj_d[:])
            # aug rows: kT2 row 64+r of head h is 1.0 iff r == h;
            # qT2 row 64+r of every head = -slope_r * i
            nc.sync.dma_start(kT2[64:72, :, :], kaug_d[:])
            nc.sync.dma_start(qT2[64:72, :, :], qaug_d[:])

            # ---- projections: qT, kT, v in one x stream ----
            with (
                tc.tile_pool(name="wqk", bufs=1) as wqk,
                tc.tile_pool(name="wst", bufs=2) as wst,
                tc.tile_pool(name="xp1", bufs=2) as xp1,
                tc.tile_pool(name="xr1", bufs=10) as xr1,
            ):
                wqr = wqk.tile([P, 8, 512], bf16)
                wkr = wqk.tile([P, 8, 512], bf16)
                wvr = wqk.tile([P, 8, 512], bf16)
                for w_dram, w_rnd in ((wq_r, wqr), (wk_r, wkr), (wv_r, wvr)):
                    for c in range(8):
                        w32 = wst.tile([P, 512], f32, tag="w32")
                        nc.sync.dma_start(w32[:], w_dram[:, c, :])
                        nc.vector.tensor_copy(w_rnd[:, c, :], w32[:])

                for tck in range(NCH):  # t-chunks of 512
                    xts = []
                    for c in range(8):
                        x32 = xp1.tile([P, 512], f32, tag="x32")
                        nc.sync.dma_start(x32[:], xT_r[:, c, bass.ts(tck, 512)])
                        xtr = xr1.tile([P, 512], bf16, tag="xtr")
                        nc.vector.tensor_copy(xtr[:], x32[:])
                        xts.append(xtr)
                    for m in range(4):
                        psq = psB.tile([P, 512], f32, tag="pb")
                        psk = psB.tile([P, 512], f32, tag="pb")
                        for c in range(8):
                            nc.tensor.matmul(
                                psq[:],
                                wqr[:, c, bass.ts(m, P)],
                                xts[c][:],
                                start=(c == 0),
                                stop=(c == 7),
                            )
                            nc.tensor.matmul(
                                psk[:],
                                wkr[:, c, bass.ts(m, P)],
                                xts[c][:],
                                start=(c == 0),
                                stop=(c == 7),
                            )
                        qstag = work.tile([P, 512], bf16, tag="qkstag")
                        kstag = work.tile([P, 512], bf16, tag="qkstag")
                        nc.vector.tensor_copy(qstag[:], psq[:])
                        nc.vector.tensor_copy(kstag[:], psk[:])
                        tsl = bass.ts(tck, 512)
                        nc.sync.dma_start(qT2[0:64, 2 * m, tsl], qstag[0:64, :])
                        nc.sync.dma_start(qT2[0:64, 2 * m + 1, tsl], qstag[64:128, :])
                        nc.sync.dma_start(kT2[0:64, 2 * m, tsl], kstag[0:64, :])
                        nc.sync.dma_start(kT2[0:64, 2 * m + 1, tsl], kstag[64:128, :])
                    for tb in range(4):
                        psv = psB.tile([P, 512], f32, tag="pb")
                        for c in range(8):
                            nc.tensor.matmul(
                                psv[:],
                                xts[c][:, bass.ts(tb, P)],
                                wvr[:, c, :],
                                start=(c == 0),
                                stop=(c == 7),
                            )
                        nc.vector.tensor_copy(
                            vaug[:, 4 * tck + tb, :, 0:64],
                            psv[:].rearrange("p (h d) -> p h d", h=NHC),
                        )

            # ---- attention ----
            ptp_cm = tc.tile_pool(name="ptp", bufs=2)
            ptp = ptp_cm.__enter__()
            for h in range(NHC):
                hp = (h % 2) * 64
                hm = h // 2
                pT = ptp.tile([P, NPT, 512], bf16, tag="pT")
                for jb in range(NJB):
                    c0 = jb // 4
                    nact = NCH - c0
                    idx0 = _PT_OFFS[jb]
                    # sub-groups of <=2 chunks so QK can run ahead of exp
                    for g0 in range(0, nact, 2):
                        ng = min(2, nact - g0)
                        ssum = psA.tile([P, 2, 512], f32, tag="ssum")
                        for ci in range(ng):
                            c = c0 + g0 + ci
                            nc.tensor.matmul(
                                ssum[:, ci, :],
                                kT2[:, h, bass.ts(jb, P)],
                                qT2[:, h, bass.ts(c, 512)],
                                start=True,
                                stop=True,
                            )
                        nc.scalar.activation(
                            pT[:, idx0 + g0 : idx0 + g0 + ng, :],
                            ssum[:, 0:ng, :],
                            mybir.ActivationFunctionType.Exp,
                            bias=biasj[:, h, jb : jb + 1],
                            scale=1.0,
                        )
                    # causal mask on the diagonal tile (c == c0):
                    # keep where i - j >= 0 ; i = 512*c0 + f, j = 128*jb + p
                    nc.gpsimd.affine_select(
                        pT[:, idx0, :],
                        pT[:, idx0, :],
                        pattern=[[1, 512]],
                        compare_op=mybir.AluOpType.is_ge,
                        fill=0.0,
                        base=512 * c0 - 128 * jb,
                        channel_multiplier=-1,
                    )

                for c in range(NCH):
                    pot = psC.tile([65, 512], f32, tag="pot")
                    njb = 4 * c + 4
                    for jb in range(njb):
                        nc.tensor.matmul(
                            pot[:],
                            vaug[:, jb, h, 0:65],
                            pT[:, _PT_OFFS[jb] + (c - jb // 4), :],
                            start=(jb == 0),
                            stop=(jb == njb - 1),
                        )
                    # copy out fast to release the PSUM bank, then normalize
                    # off the PV critical path with a broadcast + divide.
                    potsb = work.tile([65, 512], f32, tag="potsb")
                    nc.vector.tensor_copy(potsb[:], pot[:])
                    srecip = work.tile([1, 512], f32, tag="srecip")
                    nc.vector.reciprocal(srecip[:], potsb[64:65, :])
                    bcast = work.tile([64, 512], f32, tag="bcast")
                    nc.gpsimd.partition_broadcast(bcast[:], srecip[:])
                    nc.vector.tensor_tensor(
                        oT[hp : hp + 64, hm, bass.ts(c, 512)],
                        potsb[0:64, :],
                        bcast[:],
                        mybir.AluOpType.mult,
                    )

            ptp_cm.__exit__(None, None, None)

            # ---- output projection ----
            with (
                tc.tile_pool(name="wop", bufs=1) as wop,
                tc.tile_pool(name="wst3", bufs=2) as wst3,
            ):
                wor = wop.tile([P, 4, C], bf16)
                for m in range(4):
                    wo32 = wst3.tile([P, C], f32, tag="wo32")
                    nc.sync.dma_start(wo32[:], wo_r[:, m, :])
                    nc.vector.tensor_copy(wor[:, m, :], wo32[:])

                for tb in range(NJB):
                    for cc in range(2):
                        psy = psB.tile([P, 512], f32, tag="pb")
                        for m in range(4):
                            nc.tensor.matmul(
                                psy[:],
                                oT[:, m, bass.ts(tb, P)],
                                wor[:, m, bass.ts(cc, 512)],
                                start=(m == 0),
                                stop=(m == 3),
                            )
                        ysb = work.tile([P, 512], f32, tag="ysb")
                        nc.vector.tensor_copy(ysb[:], psy[:])
                        nc.sync.dma_start(y_r[:, tb, bass.ts(cc, 512)], ysb[:])

    nc.compile()
    return nc


def kernel(x, Wq, Wk, Wv, Wo):
    global LAST_RESULTS, _NC_CACHE
    x = np.asarray(x, dtype=np.float32)
    Wq = np.asarray(Wq, dtype=np.float32)
    Wk = np.asarray(Wk, dtype=np.float32)
    Wv = np.asarray(Wv, dtype=np.float32)
    Wo = np.asarray(Wo, dtype=np.float32)

    slopes = np.asarray(get_slopes(NH), dtype=np.float32)
    ii = np.arange(T, dtype=np.float64)
    pp = np.arange(P, dtype=np.float64)

    if _NC_CACHE is None:
        _NC_CACHE = build_kernel()
    nc = _NC_CACHE

    in_maps = []
    for core in range(8):
        b, g = core // 2, core % 2
        hsl = slice(g * 512, (g + 1) * 512)
        core_slopes = slopes[g * NHC : (g + 1) * NHC].astype(np.float64)
        import ml_dtypes

        qaug1 = (-core_slopes[:, None] * ii[None, :]).astype(ml_dtypes.bfloat16)
        qaugb = np.ascontiguousarray(
            np.broadcast_to(qaug1[:, None, :], (8, NHC, T))
        )
        kaugb = np.zeros((8, NHC, T), ml_dtypes.bfloat16)
        for h in range(NHC):
            kaugb[h, h, :] = ml_dtypes.bfloat16(1.0)
        biasj = np.zeros((P, NHC, NJB), np.float32)
        for h in range(NHC):
            for jb in range(NJB):
                biasj[:, h, jb] = (core_slopes[h] * (128 * jb + pp)).astype(np.float32)
        in_maps.append(
            {
                "xT": np.ascontiguousarray(x[b].T),
                "wq": np.ascontiguousarray(Wq[:, hsl]) * np.float32(0.125),
                "wk": np.ascontiguousarray(Wk[:, hsl]),
                "wv": np.ascontiguousarray(Wv[:, hsl]),
                "wo": np.ascontiguousarray(Wo[hsl, :]),
                "qaugb": qaugb,
                "kaugb": kaugb,
                "biasj": biasj,
            }
        )

    res = run_bass_kernel_spmd(nc, in_maps, list(range(8)))
    LAST_RESULTS = res
    out = np.empty((B, T, C), dtype=np.float32)
    for b in range(B):
        out[b] = res.results[2 * b]["y"] + res.results[2 * b + 1]["y"]
    return out


# revision 35
# speedup vs baseline: 2.1059x; 1.1418x over previous
"""Causal attention with ALiBi for nn_CausalAttention (B=4, T=2048, C=1024,
16 heads) on 8 TRN2 NeuronCores.

Sharding: batch (4) x head-group (2 groups of 8 heads) -> 8 cores.
Each core computes, for its batch b and head group g:
  qT/kT = (Wg.T @ x.T) projections in [d, t] layout, v in [t, d] layout,
  per head: sT[j, i] = qk/8 + slope*(j - i) via an augmented one-hot bias
  matmul (per-column -slope*i, numerically cancels in softmax) plus an ACT
  exp bias of +slope*j (exact fp32); causal masking by gpsimd affine_select
  (also kills Inf from masked overflow); PV with an appended ones column
  gives the softmax denominator; normalization via vector.reciprocal +
  gpsimd partition_broadcast; final y_partial = oT.T @ Wo_rows.
Host sums the two head-group partials per batch.

Matmuls run in float32r (TF32-like, ~1e-3 rel); probs/V in bf16.
"""

import math

import numpy as np

import concourse.bass as bass
import concourse.mybir as mybir
import concourse.tile as tile
from concourse import bacc
from concourse.bass_utils import run_bass_kernel_spmd

B, T, C = 4, 2048, 1024
NH, HD = 16, 64
NHC = 8  # heads per core
BLOCK_SIZE = 2048
NJB = T // 128  # 16 j-blocks
NCH = T // 512  # 4 i-chunks
P = 128

f32 = mybir.dt.float32
f32r = mybir.dt.float32r
bf16 = mybir.dt.bfloat16

LAST_RESULTS = None
_NC_CACHE = None


def get_slopes(n):
    def pow2(n):
        start = 2 ** (-(2 ** (-(math.log2(n) - 3))))
        return [start * start**i for i in range(n)]

    if math.log2(n).is_integer():
        return pow2(n)
    c = 2 ** math.floor(math.log2(n))
    return pow2(c) + get_slopes(2 * c)[0::2][: n - c]


# compact pT tile index: tiles (jb, c) with c >= jb//4
_PT_OFFS = []
_o = 0
for _jb in range(NJB):
    _PT_OFFS.append(_o)
    _o += NCH - _jb // 4
NPT = _o  # 40


def build_kernel():
    nc = bacc.Bacc("TRN2", target_bir_lowering=False, debug=False, num_devices=8)

    xT_d = nc.dram_tensor("xT", [C, T], f32, kind="ExternalInput").ap()
    wq_d = nc.dram_tensor("wq", [C, 512], f32, kind="ExternalInput").ap()
    wk_d = nc.dram_tensor("wk", [C, 512], f32, kind="ExternalInput").ap()
    wv_d = nc.dram_tensor("wv", [C, 512], f32, kind="ExternalInput").ap()
    wo_d = nc.dram_tensor("wo", [512, C], f32, kind="ExternalInput").ap()
    qaug_d = nc.dram_tensor("qaugb", [8, NHC, T], bf16, kind="ExternalInput").ap()
    kaug_d = nc.dram_tensor("kaugb", [8, NHC, T], bf16, kind="ExternalInput").ap()
    biasj_d = nc.dram_tensor("biasj", [P, NHC, NJB], f32, kind="ExternalInput").ap()
    y_d = nc.dram_tensor("y", [T, C], f32, kind="ExternalOutput").ap()

    xT_r = xT_d.rearrange("(cb p) t -> p cb t", p=P)  # [128, 8, 2048]
    wq_r = wq_d.rearrange("(cb p) m -> p cb m", p=P)  # [128, 8, 512]
    wk_r = wk_d.rearrange("(cb p) m -> p cb m", p=P)
    wv_r = wv_d.rearrange("(cb p) m -> p cb m", p=P)
    wo_r = wo_d.rearrange("(mb p) n -> p mb n", p=P)  # [128, 4, 1024]
    y_r = y_d.rearrange("(tb p) c -> p tb c", p=P)  # [128, 16, 1024]

    with tile.TileContext(nc) as tc:
        with (
            tc.tile_pool(name="persist", bufs=1) as persist,
            tc.tile_pool(name="work", bufs=2) as work,
            tc.tile_pool(name="psA", bufs=2, space="PSUM") as psA,
            tc.tile_pool(name="psB", bufs=2, space="PSUM") as psB,
            tc.tile_pool(name="psC", bufs=2, space="PSUM") as psC,
        ):
            # ---- persistent tiles ----
            # qT2/kT2: per head h, rows 0-63 = head data (d), rows 64-71 =
            # augmented bias rows; K=72 matmul contracts both at once.
            qT2 = persist.tile([72, NHC, T], bf16)
            kT2 = persist.tile([72, NHC, T], bf16)
            vaug = persist.tile([P, NJB, NHC, 66], bf16)
            oT = persist.tile([P, 4, T], bf16)
            biasj = persist.tile([P, NHC, NJB], f32)

            nc.gpsimd.memset(vaug[:, :, :, 64:66], 1.0)
            nc.sync.dma_start(biasj[:], biasj_d[:])
            # aug rows: kT2 row 64+r of head h is 1.0 iff r == h;
            # qT2 row 64+r of every head = -slope_r * i
            nc.sync.dma_start(kT2[64:72, :, :], kaug_d[:])
            nc.sync.dma_start(qT2[64:72, :, :], qaug_d[:])

            # ---- projections: qT, kT, v in one x stream ----
            with (
                tc.tile_pool(name="wqk", bufs=1) as wqk,
                tc.tile_pool(name="wst", bufs=2) as wst,
                tc.tile_pool(name="xp1", bufs=2) as xp1,
                tc.tile_pool(name="xr1", bufs=10) as xr1,
            ):
                wqr = wqk.tile([P, 8, 512], bf16)
                wkr = wqk.tile([P, 8, 512], bf16)
                wvr = wqk.tile([P, 8, 512], bf16)
                for w_dram, w_rnd in ((wq_r, wqr), (wk_r, wkr), (wv_r, wvr)):
                    for c in range(8):
                        w32 = wst.tile([P, 512], f32, tag="w32")
                        nc.sync.dma_start(w32[:], w_dram[:, c, :])
                        nc.vector.tensor_copy(w_rnd[:, c, :], w32[:])

                for tck in range(NCH):  # t-chunks of 512
                    xts = []
                    for c in range(8):
                        x32 = xp1.tile([P, 512], f32, tag="x32")
                        nc.sync.dma_start(x32[:], xT_r[:, c, bass.ts(tck, 512)])
                        xtr = xr1.tile([P, 512], bf16, tag="xtr")
                        nc.vector.tensor_copy(xtr[:], x32[:])
                        xts.append(xtr)
                    for m in range(4):
                        psq = psB.tile([P, 512], f32, tag="pb")
                        psk = psB.tile([P, 512], f32, tag="pb")
                        for c in range(8):
                            nc.tensor.matmul(
                                psq[:],
                                wqr[:, c, bass.ts(m, P)],
                                xts[c][:],
                                start=(c == 0),
                                stop=(c == 7),
                            )
                            nc.tensor.matmul(
                                psk[:],
                                wkr[:, c, bass.ts(m, P)],
                                xts[c][:],
                                start=(c == 0),
                                stop=(c == 7),
                            )
                        qstag = work.tile([P, 512], bf16, tag="qkstag")
                        kstag = work.tile([P, 512], bf16, tag="qkstag")
                        nc.vector.tensor_copy(qstag[:], psq[:])
                        nc.vector.tensor_copy(kstag[:], psk[:])
                        tsl = bass.ts(tck, 512)
                        nc.sync.dma_start(qT2[0:64, 2 * m, tsl], qstag[0:64, :])
                        nc.sync.dma_start(qT2[0:64, 2 * m + 1, tsl], qstag[64:128, :])
                        nc.sync.dma_start(kT2[0:64, 2 * m, tsl], kstag[0:64, :])
                        nc.sync.dma_start(kT2[0:64, 2 * m + 1, tsl], kstag[64:128, :])
                    for tb in range(4):
                        psv = psB.tile([P, 512], f32, tag="pb")
                        for c in range(8):
                            nc.tensor.matmul(
                                psv[:],
                                xts[c][:, bass.ts(tb, P)],
                                wvr[:, c, :],
                                start=(c == 0),
                                stop=(c == 7),
                            )
                        nc.vector.tensor_copy(
                            vaug[:, 4 * tck + tb, :, 0:64],
                            psv[:].rearrange("p (h d) -> p h d", h=NHC),
                        )

            # ---- attention ----
            ptp_cm = tc.tile_pool(name="ptp", bufs=2)
            ptp = ptp_cm.__enter__()
            for h in range(NHC):
                hp = (h % 2) * 64
                hm = h // 2
                pT = ptp.tile([P, NPT, 512], bf16, tag="pT")
                for jb in range(NJB):
                    c0 = jb // 4
                    nact = NCH - c0
                    idx0 = _PT_OFFS[jb]
                    # sub-groups of <=2 chunks so QK can run ahead of exp
                    for g0 in range(0, nact, 2):
                        ng = min(2, nact - g0)
                        ssum = psA.tile([P, 2, 512], f32, tag="ssum")
                        for ci in range(ng):
                            c = c0 + g0 + ci
                            nc.tensor.matmul(
                                ssum[:, ci, :],
                                kT2[:, h, bass.ts(jb, P)],
                                qT2[:, h, bass.ts(c, 512)],
                                start=True,
                                stop=True,
                            )
                        nc.scalar.activation(
                            pT[:, idx0 + g0 : idx0 + g0 + ng, :],
                            ssum[:, 0:ng, :],
                            mybir.ActivationFunctionType.Exp,
                            bias=biasj[:, h, jb : jb + 1],
                            scale=1.0,
                        )
                    # causal mask on the diagonal tile (c == c0):
                    # keep where i - j >= 0 ; i = 512*c0 + f, j = 128*jb + p
                    nc.gpsimd.affine_select(
                        pT[:, idx0, :],
                        pT[:, idx0, :],
                        pattern=[[1, 512]],
                        compare_op=mybir.AluOpType.is_ge,
                        fill=0.0,
                        base=512 * c0 - 128 * jb,
                        channel_multiplier=-1,
                    )

                for c in range(NCH):
                    pot = psC.tile([65, 512], f32, tag="pot")
                    njb = 4 * c + 4
                    for jb in range(njb):
                        nc.tensor.matmul(
                            pot[:],
                            vaug[:, jb, h, 0:65],
                            pT[:, _PT_OFFS[jb] + (c - jb // 4), :],
                            start=(jb == 0),
                            stop=(jb == njb - 1),
                        )
                    # copy out fast to release the PSUM bank, then normalize
                    # off the PV critical path with a broadcast + divide.
                    potsb = work.tile([65, 512], f32, tag="potsb")
                    nc.vector.tensor_copy(potsb[:], pot[:])
                    # spread the 512 rowsums across 128 partitions so the
                    # reciprocal uses all DVE lanes (26ns vs 3.3us)
                    rs128 = work.tile([P, 4], f32, tag="rs128")
                    nc.sync.dma_start(rs128[:], potsb[64:65, :])
                    nc.vector.reciprocal(rs128[:], rs128[:])
                    srecip = work.tile([1, 512], f32, tag="srecip")
                    nc.sync.dma_start(srecip[:], rs128[:])
                    bcast = work.tile([64, 512], f32, tag="bcast")
                    nc.gpsimd.partition_broadcast(bcast[:], srecip[:])
                    nc.vector.tensor_tensor(
                        oT[hp : hp + 64, hm, bass.ts(c, 512)],
                        potsb[0:64, :],
                        bcast[:],
                        mybir.AluOpType.mult,
                    )

            ptp_cm.__exit__(None, None, None)

            # ---- output projection ----
            with (
                tc.tile_pool(name="wop", bufs=1) as wop,
                tc.tile_pool(name="wst3", bufs=2) as wst3,
            ):
                wor = wop.tile([P, 4, C], bf16)
                for m in range(4):
                    wo32 = wst3.tile([P, C], f32, tag="wo32")
                    nc.sync.dma_start(wo32[:], wo_r[:, m, :])
                    nc.vector.tensor_copy(wor[:, m, :], wo32[:])

                for tb in range(NJB):
                    for cc in range(2):
                        psy = psB.tile([P, 512], f32, tag="pb")
                        for m in range(4):
                            nc.tensor.matmul(
                                psy[:],
                                oT[:, m, bass.ts(tb, P)],
                                wor[:, m, bass.ts(cc, 512)],
                                start=(m == 0),
                                stop=(m == 3),
                            )
                        ysb = work.tile([P, 512], f32, tag="ysb")
                        nc.vector.tensor_copy(ysb[:], psy[:])
                        nc.sync.dma_start(y_r[:, tb, bass.ts(cc, 512)], ysb[:])

    nc.compile()
    return nc


def kernel(x, Wq, Wk, Wv, Wo):
    global LAST_RESULTS, _NC_CACHE
    x = np.asarray(x, dtype=np.float32)
    Wq = np.asarray(Wq, dtype=np.float32)
    Wk = np.asarray(Wk, dtype=np.float32)
    Wv = np.asarray(Wv, dtype=np.float32)
    Wo = np.asarray(Wo, dtype=np.float32)

    slopes = np.asarray(get_slopes(NH), dtype=np.float32)
    ii = np.arange(T, dtype=np.float64)
    pp = np.arange(P, dtype=np.float64)

    if _NC_CACHE is None:
        _NC_CACHE = build_kernel()
    nc = _NC_CACHE

    in_maps = []
    for core in range(8):
        b, g = core // 2, core % 2
        hsl = slice(g * 512, (g + 1) * 512)
        core_slopes = slopes[g * NHC : (g + 1) * NHC].astype(np.float64)
        import ml_dtypes

        qaug1 = (-core_slopes[:, None] * ii[None, :]).astype(ml_dtypes.bfloat16)
        qaugb = np.ascontiguousarray(
            np.broadcast_to(qaug1[:, None, :], (8, NHC, T))
        )
        kaugb = np.zeros((8, NHC, T), ml_dtypes.bfloat16)
        for h in range(NHC):
            kaugb[h, h, :] = ml_dtypes.bfloat16(1.0)
        biasj = np.zeros((P, NHC, NJB), np.float32)
        for h in range(NHC):
            for jb in range(NJB):
                biasj[:, h, jb] = (core_slopes[h] * (128 * jb + pp)).astype(np.float32)
        in_maps.append(
            {
                "xT": np.ascontiguousarray(x[b].T),
                "wq": np.ascontiguousarray(Wq[:, hsl]) * np.float32(0.125),
                "wk": np.ascontiguousarray(Wk[:, hsl]),
                "wv": np.ascontiguousarray(Wv[:, hsl]),
                "wo": np.ascontiguousarray(Wo[hsl, :]),
                "qaugb": qaugb,
                "kaugb": kaugb,
                "biasj": biasj,
            }
        )

    res = run_bass_kernel_spmd(nc, in_maps, list(range(8)))
    LAST_RESULTS = res
    out = np.empty((B, T, C), dtype=np.float32)
    for b in range(B):
        out[b] = res.results[2 * b]["y"] + res.results[2 * b + 1]["y"]
    return out


# revision 36
# speedup vs baseline: 2.1438x; 1.0180x over previous
"""Causal attention with ALiBi for nn_CausalAttention (B=4, T=2048, C=1024,
16 heads) on 8 TRN2 NeuronCores.

Sharding: batch (4) x head-group (2 groups of 8 heads) -> 8 cores.
Each core computes, for its batch b and head group g:
  qT/kT = (Wg.T @ x.T) projections in [d, t] layout, v in [t, d] layout,
  per head: sT[j, i] = qk/8 + slope*(j - i) via an augmented one-hot bias
  matmul (per-column -slope*i, numerically cancels in softmax) plus an ACT
  exp bias of +slope*j (exact fp32); causal masking by gpsimd affine_select
  (also kills Inf from masked overflow); PV with an appended ones column
  gives the softmax denominator; normalization via vector.reciprocal +
  gpsimd partition_broadcast; final y_partial = oT.T @ Wo_rows.
Host sums the two head-group partials per batch.

Matmuls run in float32r (TF32-like, ~1e-3 rel); probs/V in bf16.
"""

import math

import numpy as np

import concourse.bass as bass
import concourse.mybir as mybir
import concourse.tile as tile
from concourse import bacc
from concourse.bass_utils import run_bass_kernel_spmd

B, T, C = 4, 2048, 1024
NH, HD = 16, 64
NHC = 8  # heads per core
BLOCK_SIZE = 2048
NJB = T // 128  # 16 j-blocks
NCH = T // 512  # 4 i-chunks
P = 128

f32 = mybir.dt.float32
f32r = mybir.dt.float32r
bf16 = mybir.dt.bfloat16

LAST_RESULTS = None
_NC_CACHE = None


def get_slopes(n):
    def pow2(n):
        start = 2 ** (-(2 ** (-(math.log2(n) - 3))))
        return [start * start**i for i in range(n)]

    if math.log2(n).is_integer():
        return pow2(n)
    c = 2 ** math.floor(math.log2(n))
    return pow2(c) + get_slopes(2 * c)[0::2][: n - c]


# compact pT tile index: tiles (jb, c) with c >= jb//4
_PT_OFFS = []
_o = 0
for _jb in range(NJB):
    _PT_OFFS.append(_o)
    _o += NCH - _jb // 4
NPT = _o  # 40


def build_kernel():
    nc = bacc.Bacc("TRN2", target_bir_lowering=False, debug=False, num_devices=8)

    xT_d = nc.dram_tensor("xT", [C, T], f32, kind="ExternalInput").ap()
    wq_d = nc.dram_tensor("wq", [C, 512], f32, kind="ExternalInput").ap()
    wk_d = nc.dram_tensor("wk", [C, 512], f32, kind="ExternalInput").ap()
    wv_d = nc.dram_tensor("wv", [C, 512], f32, kind="ExternalInput").ap()
    wo_d = nc.dram_tensor("wo", [512, C], f32, kind="ExternalInput").ap()
    qaug_d = nc.dram_tensor("qaugb", [8, NHC, T], bf16, kind="ExternalInput").ap()
    kaug_d = nc.dram_tensor("kaugb", [8, NHC, T], bf16, kind="ExternalInput").ap()
    biasj_d = nc.dram_tensor("biasj", [P, NHC, NJB], f32, kind="ExternalInput").ap()
    y_d = nc.dram_tensor("y", [T, C], f32, kind="ExternalOutput").ap()

    xT_r = xT_d.rearrange("(cb p) t -> p cb t", p=P)  # [128, 8, 2048]
    wq_r = wq_d.rearrange("(cb p) m -> p cb m", p=P)  # [128, 8, 512]
    wk_r = wk_d.rearrange("(cb p) m -> p cb m", p=P)
    wv_r = wv_d.rearrange("(cb p) m -> p cb m", p=P)
    wo_r = wo_d.rearrange("(mb p) n -> p mb n", p=P)  # [128, 4, 1024]
    y_r = y_d.rearrange("(tb p) c -> p tb c", p=P)  # [128, 16, 1024]

    with tile.TileContext(nc) as tc:
        with (
            tc.tile_pool(name="persist", bufs=1) as persist,
            tc.tile_pool(name="work", bufs=2) as work,
            tc.tile_pool(name="psA", bufs=2, space="PSUM") as psA,
            tc.tile_pool(name="psB", bufs=2, space="PSUM") as psB,
            tc.tile_pool(name="psC", bufs=2, space="PSUM") as psC,
        ):
            # ---- persistent tiles ----
            # qT2/kT2: per head h, rows 0-63 = head data (d), rows 64-71 =
            # augmented bias rows; K=72 matmul contracts both at once.
            qT2 = persist.tile([72, NHC, T], bf16)
            kT2 = persist.tile([72, NHC, T], bf16)
            vaug = persist.tile([P, NJB, NHC, 66], bf16)
            oT = persist.tile([P, 4, T], bf16)
            biasj = persist.tile([P, NHC, NJB], f32)

            nc.gpsimd.memset(vaug[:, :, :, 64:66], 1.0)
            nc.sync.dma_start(biasj[:], biasj_d[:])
            # aug rows: kT2 row 64+r of head h is 1.0 iff r == h;
            # qT2 row 64+r of every head = -slope_r * i
            nc.sync.dma_start(kT2[64:72, :, :], kaug_d[:])
            nc.sync.dma_start(qT2[64:72, :, :], qaug_d[:])

            # ---- projections: qT, kT, v in one x stream ----
            with (
                tc.tile_pool(name="wqk", bufs=1) as wqk,
                tc.tile_pool(name="wst", bufs=2) as wst,
                tc.tile_pool(name="xp1", bufs=2) as xp1,
                tc.tile_pool(name="xr1", bufs=10) as xr1,
            ):
                wqr = wqk.tile([P, 8, 512], bf16)
                wkr = wqk.tile([P, 8, 512], bf16)
                wvr = wqk.tile([P, 8, 512], bf16)
                for w_dram, w_rnd in ((wq_r, wqr), (wk_r, wkr), (wv_r, wvr)):
                    for c in range(8):
                        w32 = wst.tile([P, 512], f32, tag="w32")
                        nc.sync.dma_start(w32[:], w_dram[:, c, :])
                        nc.vector.tensor_copy(w_rnd[:, c, :], w32[:])

                for tck in range(NCH):  # t-chunks of 512
                    xts = []
                    for c in range(8):
                        x32 = xp1.tile([P, 512], f32, tag="x32")
                        nc.sync.dma_start(x32[:], xT_r[:, c, bass.ts(tck, 512)])
                        xtr = xr1.tile([P, 512], bf16, tag="xtr")
                        nc.vector.tensor_copy(xtr[:], x32[:])
                        xts.append(xtr)
                    for m in range(4):
                        psq = psB.tile([P, 512], f32, tag="pb")
                        psk = psB.tile([P, 512], f32, tag="pb")
                        for c in range(8):
                            nc.tensor.matmul(
                                psq[:],
                                wqr[:, c, bass.ts(m, P)],
                                xts[c][:],
                                start=(c == 0),
                                stop=(c == 7),
                            )
                            nc.tensor.matmul(
                                psk[:],
                                wkr[:, c, bass.ts(m, P)],
                                xts[c][:],
                                start=(c == 0),
                                stop=(c == 7),
                            )
                        qstag = work.tile([P, 512], bf16, tag="qkstag")
                        kstag = work.tile([P, 512], bf16, tag="qkstag")
                        nc.vector.tensor_copy(qstag[:], psq[:])
                        nc.vector.tensor_copy(kstag[:], psk[:])
                        tsl = bass.ts(tck, 512)
                        nc.sync.dma_start(qT2[0:64, 2 * m, tsl], qstag[0:64, :])
                        nc.sync.dma_start(qT2[0:64, 2 * m + 1, tsl], qstag[64:128, :])
                        nc.sync.dma_start(kT2[0:64, 2 * m, tsl], kstag[0:64, :])
                        nc.sync.dma_start(kT2[0:64, 2 * m + 1, tsl], kstag[64:128, :])
                    for tb in range(4):
                        psv = psB.tile([P, 512], f32, tag="pb")
                        for c in range(8):
                            nc.tensor.matmul(
                                psv[:],
                                xts[c][:, bass.ts(tb, P)],
                                wvr[:, c, :],
                                start=(c == 0),
                                stop=(c == 7),
                            )
                        nc.vector.tensor_copy(
                            vaug[:, 4 * tck + tb, :, 0:64],
                            psv[:].rearrange("p (h d) -> p h d", h=NHC),
                        )

            # ---- attention (software-pipelined: QK(h) runs while the
            # exp/select tail + PV of head h-1 drain) ----
            ptp_cm = tc.tile_pool(name="ptp", bufs=2)
            ptp = ptp_cm.__enter__()
            pT_of = {}

            def emit_qk(h):
                pT = ptp.tile([P, NPT, 512], bf16, tag="pT")
                pT_of[h] = pT
                for jb in range(NJB):
                    c0 = jb // 4
                    nact = NCH - c0
                    idx0 = _PT_OFFS[jb]
                    # sub-groups of <=2 chunks so QK can run ahead of exp
                    for g0 in range(0, nact, 2):
                        ng = min(2, nact - g0)
                        ssum = psA.tile([P, 2, 512], f32, tag="ssum")
                        for ci in range(ng):
                            c = c0 + g0 + ci
                            nc.tensor.matmul(
                                ssum[:, ci, :],
                                kT2[:, h, bass.ts(jb, P)],
                                qT2[:, h, bass.ts(c, 512)],
                                start=True,
                                stop=True,
                            )
                        nc.scalar.activation(
                            pT[:, idx0 + g0 : idx0 + g0 + ng, :],
                            ssum[:, 0:ng, :],
                            mybir.ActivationFunctionType.Exp,
                            bias=biasj[:, h, jb : jb + 1],
                            scale=1.0,
                        )
                    # causal mask on the diagonal tile (c == c0):
                    # keep where i - j >= 0 ; i = 512*c0 + f, j = 128*jb + p
                    nc.gpsimd.affine_select(
                        pT[:, idx0, :],
                        pT[:, idx0, :],
                        pattern=[[1, 512]],
                        compare_op=mybir.AluOpType.is_ge,
                        fill=0.0,
                        base=512 * c0 - 128 * jb,
                        channel_multiplier=-1,
                    )

            def emit_pv(h):
                hp = (h % 2) * 64
                hm = h // 2
                pT = pT_of.pop(h)
                for c in range(NCH):
                    pot = psC.tile([65, 512], f32, tag="pot")
                    njb = 4 * c + 4
                    for jb in range(njb):
                        nc.tensor.matmul(
                            pot[:],
                            vaug[:, jb, h, 0:65],
                            pT[:, _PT_OFFS[jb] + (c - jb // 4), :],
                            start=(jb == 0),
                            stop=(jb == njb - 1),
                        )
                    # copy out fast to release the PSUM bank, then normalize
                    # off the PV critical path.
                    potsb = work.tile([65, 512], f32, tag="potsb")
                    nc.vector.tensor_copy(potsb[:], pot[:])
                    # spread the 512 rowsums across 128 partitions so the
                    # reciprocal uses all DVE lanes (26ns vs 3.3us)
                    rs128 = work.tile([P, 4], f32, tag="rs128")
                    nc.sync.dma_start(rs128[:], potsb[64:65, :])
                    nc.vector.reciprocal(rs128[:], rs128[:])
                    srecip = work.tile([1, 512], f32, tag="srecip")
                    nc.sync.dma_start(srecip[:], rs128[:])
                    bcast = work.tile([64, 512], f32, tag="bcast")
                    nc.gpsimd.partition_broadcast(bcast[:], srecip[:])
                    nc.vector.tensor_tensor(
                        oT[hp : hp + 64, hm, bass.ts(c, 512)],
                        potsb[0:64, :],
                        bcast[:],
                        mybir.AluOpType.mult,
                    )

            for h in range(NHC + 1):
                if h < NHC:
                    emit_qk(h)
                if h >= 1:
                    emit_pv(h - 1)

            ptp_cm.__exit__(None, None, None)

            # ---- output projection ----
            with (
                tc.tile_pool(name="wop", bufs=1) as wop,
                tc.tile_pool(name="wst3", bufs=2) as wst3,
            ):
                wor = wop.tile([P, 4, C], bf16)
                for m in range(4):
                    wo32 = wst3.tile([P, C], f32, tag="wo32")
                    nc.sync.dma_start(wo32[:], wo_r[:, m, :])
                    nc.vector.tensor_copy(wor[:, m, :], wo32[:])

                for tb in range(NJB):
                    for cc in range(2):
                        psy = psB.tile([P, 512], f32, tag="pb")
                        for m in range(4):
                            nc.tensor.matmul(
                                psy[:],
                                oT[:, m, bass.ts(tb, P)],
                                wor[:, m, bass.ts(cc, 512)],
                                start=(m == 0),
                                stop=(m == 3),
                            )
                        ysb = work.tile([P, 512], f32, tag="ysb")
                        nc.vector.tensor_copy(ysb[:], psy[:])
                        nc.sync.dma_start(y_r[:, tb, bass.ts(cc, 512)], ysb[:])

    nc.compile()
    return nc


def kernel(x, Wq, Wk, Wv, Wo):
    global LAST_RESULTS, _NC_CACHE
    x = np.asarray(x, dtype=np.float32)
    Wq = np.asarray(Wq, dtype=np.float32)
    Wk = np.asarray(Wk, dtype=np.float32)
    Wv = np.asarray(Wv, dtype=np.float32)
    Wo = np.asarray(Wo, dtype=np.float32)

    slopes = np.asarray(get_slopes(NH), dtype=np.float32)
    ii = np.arange(T, dtype=np.float64)
    pp = np.arange(P, dtype=np.float64)

    if _NC_CACHE is None:
        _NC_CACHE = build_kernel()
    nc = _NC_CACHE

    in_maps = []
    for core in range(8):
        b, g = core // 2, core % 2
        hsl = slice(g * 512, (g + 1) * 512)
        core_slopes = slopes[g * NHC : (g + 1) * NHC].astype(np.float64)
        import ml_dtypes

        qaug1 = (-core_slopes[:, None] * ii[None, :]).astype(ml_dtypes.bfloat16)
        qaugb = np.ascontiguousarray(
            np.broadcast_to(qaug1[:, None, :], (8, NHC, T))
        )
        kaugb = np.zeros((8, NHC, T), ml_dtypes.bfloat16)
        for h in range(NHC):
            kaugb[h, h, :] = ml_dtypes.bfloat16(1.0)
        biasj = np.zeros((P, NHC, NJB), np.float32)
        for h in range(NHC):
            for jb in range(NJB):
                biasj[:, h, jb] = (core_slopes[h] * (128 * jb + pp)).astype(np.float32)
        in_maps.append(
            {
                "xT": np.ascontiguousarray(x[b].T),
                "wq": np.ascontiguousarray(Wq[:, hsl]) * np.float32(0.125),
                "wk": np.ascontiguousarray(Wk[:, hsl]),
                "wv": np.ascontiguousarray(Wv[:, hsl]),
                "wo": np.ascontiguousarray(Wo[hsl, :]),
                "qaugb": qaugb,
                "kaugb": kaugb,
                "biasj": biasj,
            }
        )

    res = run_bass_kernel_spmd(nc, in_maps, list(range(8)))
    LAST_RESULTS = res
    out = np.empty((B, T, C), dtype=np.float32)
    for b in range(B):
        out[b] = res.results[2 * b]["y"] + res.results[2 * b + 1]["y"]
    return out
